# revision 1
# baseline (speedup 1.0000x reference)
"""CapsuleLayer dynamic-routing kernel for 8 trn2 NeuronCores.

Problem: B=128, U=8, C=2048, J=32, S=16, 3 routing iterations.
  u_hat[b,c,j,s] = sum_u W[c,j,s,u] x[b,u,c]          (never materialized: 536MB)
  iter: c=softmax(b over C); s=sum_c c*u_hat; v=squash(s); b+=mean_b(u_hat.v)

Sharding: input capsules C split 8 ways (256/core). Per iteration each core
computes s-partials over its C-slice as matmuls (contraction (u,c_loc)=2048
against a c-scaled W), one AllReduce combines s-partials + softmax
denominators, then squash/b-update are local:
  s_un[b,(j,s)]  = sum_{u,c_loc} x[(u,c),b] * (e[c,j]*W[(u,c),(j,s)])
  A[c,(j,s)]     = sum_b x[b,(u,c)] * v[b,(j,s)]     (per-u matmuls)
  b[c,j]        += (1/B) sum_{u,s} W * A
All cores end with the full (identical) v, so core 0's output is the answer.

Perf notes (vs the 196us f32r baseline; single-shot NTFF active time
~174us): bf16 datapath throughout (fp16 on the AllReduce wire, fp32
PSUM/output); b-update as ONE big contiguous tensor_tensor + one
XY tensor_reduce per c-range instead of 128 small affine_mul_reduce
calls (the DVE runs plain bf16 multiplies at ~0.56ns/elem/lane but
custom/small/broadcast ops at >1ns + ~300ns/instr overhead); A drained
PSUM->bf16 by the Activation engine; per-cr b-update chains e/dpart/ww
immediately after b+= so cr0's s-matmuls start while cr1's update is
still queued; ACT exp/sqrt table loads (1.3us each) pre-warmed by dummy
ops issued right after the collective trigger; post-collective DMAs kept
off the ACT queue so those dummies actually run during the AllReduce;
full-tile DMAs only (partition-sliced DMA APs hit a ~6x slower path).
Tried and rejected: AllGather+local sum (15us, no better than AR at this
size), dual-PSUM-bank s-accumulation (no change), gpsimd for any big
elementwise work (~3.5ns/elem, 6x slower than DVE), gpsimd
partition_broadcast (4.5us for [128,32]!).
"""

import numpy as np

B, U, C, J, S = 128, 8, 2048, 32, 16
N_CORES = 8
C_LOC = C // N_CORES          # 256
NCR = C_LOC // 128            # 2 partition-ranges per core
JS = J * S                    # 512
N_ITER = 3

_cache = {}


def _build(use_ar=True, reps=1, mmdt="bf16"):
    import concourse.bacc as bacc
    import concourse.mybir as mybir
    import concourse.tile as tile

    f32 = mybir.dt.float32
    f16 = mybir.dt.float16
    bf16 = mybir.dt.bfloat16
    mdt = bf16 if mmdt == "bf16" else mybir.dt.float32r
    AT = mybir.AluOpType
    ACT = mybir.ActivationFunctionType
    AX = mybir.AxisListType

    nc = bacc.Bacc("TRN2", target_bir_lowering=False, debug=False,
                   num_devices=N_CORES)

    # per-core inputs (host pre-sharded/transposed), cr outermost
    xs_d = nc.dram_tensor("xs", [128, NCR, U, B], mdt, kind="ExternalInput")
    xa_d = nc.dram_tensor("xa", [B, NCR, U, 128], mdt, kind="ExternalInput")
    wa_d = nc.dram_tensor("wa", [128, NCR, U, J, S], mdt, kind="ExternalInput")

    v_d = nc.dram_tensor("v", [B, JS], f32, kind="ExternalOutput")

    AR_N1 = B * JS                 # iter-1 payload: s partials only
    AR_N = B * JS + 2 * J          # iters 2-3: s partials + D partials [1,64]
    H = B * JS // 2

    with tile.TileContext(nc) as tc:
        with (
            tc.tile_pool(name="big", bufs=1) as big,
            tc.tile_pool(name="sm", bufs=2) as sm,
            tc.tile_pool(name="ps_s", bufs=1, space="PSUM") as ps_s,
            tc.tile_pool(name="ps_a", bufs=2, space="PSUM") as ps_a,
            tc.tile_pool(name="ps_t", bufs=1, space="PSUM") as ps_t,
            tc.tile_pool(name="dram", bufs=1, space="DRAM") as dram,
        ):
            # ---- resident tensors ----
            xs = big.tile([128, NCR, U, B], mdt, tag="xs")
            xa = big.tile([B, NCR, U, 128], mdt, tag="xa")
            wa = big.tile([128, NCR, U, J, S], mdt, tag="wa")
            ww = big.tile([128, NCR, U, J, S], mdt, tag="ww")
            a_sb = big.tile([128, NCR, U, J, S], mdt, tag="a_sb")
            m_sb = big.tile([128, NCR, U, J, S], mdt, tag="m_sb")

            # xs first, then wa per-u chunks alternating queues so iter-1
            # s-matmuls start early; xa (A-step) last
            nc.scalar.dma_start(xs[:], xs_d[:])
            nc.sync.dma_start(wa[:, 0], wa_d[:, 0])
            nc.gpsimd.dma_start(wa[:, 1], wa_d[:, 1])
            nc.scalar.dma_start(xa[:], xa_d[:])

            b_cr = [sm.tile([128, J], f32, tag=f"b{cr}", name=f"b{cr}")
                    for cr in range(NCR)]
            binc_cr = [sm.tile([128, J], f32, tag=f"binc{cr}", name=f"binc{cr}")
                       for cr in range(NCR)]
            r1_cr = [sm.tile([128, U, J], f32, tag=f"r1{cr}", name=f"r1{cr}")
                     for cr in range(NCR)]
            ones = sm.tile([128, 1], mdt, tag="ones")
            onesr = sm.tile([1, 128], f32, tag="onesr")
            nc.vector.memset(ones[:], 1.0)
            nc.vector.memset(onesr[:], 1.0)
            dpart = sm.tile([1, NCR * J], f16, tag="dpart")
            nc.vector.memset(dpart[:], 0.0)
            # ACT table pre-warm scratch
            dumi = sm.tile([1, 1], f32, tag="dumi")
            dumo = sm.tile([1, 1], f32, tag="dumo")
            nc.vector.memset(dumi[:], 1.0)

            for rep in range(reps):
             for it in range(N_ITER):
                first = it == 0
                last = it == N_ITER - 1

                # ---- s partials: 16 chunk matmuls into PSUM ----
                s_ps = ps_s.tile([B, JS], f32, tag="sps")
                rhs = wa if first else ww
                for cr in range(NCR):
                    for u in range(U):
                        k = cr * U + u
                        nc.tensor.matmul(
                            s_ps[:],
                            xs[:, cr, u],
                            rhs[:, cr, u].rearrange("p a b -> p (a b)"),
                            start=(k == 0), stop=(k == U * NCR - 1),
                        )
                # drain on ACT (iter1 folds the uniform 1/C softmax weight)
                s_un = sm.tile([B, JS], f16, tag="sun")
                if first:
                    nc.scalar.mul(s_un[:], s_ps[:], 1.0 / C)
                else:
                    nc.scalar.copy(s_un[:], s_ps[:])

                # ---- AllReduce: s partials (+ D partials) ----
                ar_in = dram.tile([1, AR_N], f16, tag="ar_in")
                ar_out = dram.tile([1, AR_N], f16, tag="ar_out",
                                   addr_space="Shared")
                nc.sync.dma_start(ar_in[0, 0:B * JS], s_un[:])
                if not first:
                    nc.scalar.dma_start(ar_in[0, B * JS:], dpart[:])
                if use_ar:
                    nc.gpsimd.collective_compute(
                        "AllReduce", AT.add,
                        replica_groups=[list(range(N_CORES))],
                        ins=[ar_in[:].opt()], outs=[ar_out[:].opt()],
                    )
                else:
                    nc.sync.dma_start(ar_out[:], ar_in[:])
                # pre-warm the ACT sqrt table while the AllReduce is in
                # flight so the squash sqrt doesn't eat a 1.3us table load
                nc.scalar.sqrt(dumo[:], dumi[:])
                s_sum = sm.tile([B, JS], f16, tag="ssum")

                if first:
                    # uniform-c 1/C was already folded in at the PSUM drain
                    nc.sync.dma_start(s_sum[:], ar_out[0, 0:B * JS])
                    s_t = s_sum
                else:
                    dsum = sm.tile([1, NCR * J], f16, tag="dsum")
                    nc.sync.dma_start(dsum[:], ar_out[0, B * JS:])
                    nc.gpsimd.dma_start(s_sum[:], ar_out[0, 0:B * JS])
                    # fold cr halves, reciprocal, broadcast via PE matmul
                    dfold = sm.tile([1, J], f32, tag="dfold")
                    nc.vector.tensor_add(dfold[:], dsum[:, 0:J],
                                         dsum[:, J:2 * J])
                    drec = sm.tile([1, J], f32, tag="drec")
                    nc.vector.reciprocal(drec[:], dfold[:])
                    drec_ps = ps_t.tile([128, J], f32, tag="tiny")
                    nc.tensor.matmul(drec_ps[:], onesr[:], drec[:],
                                     start=True, stop=True)
                    drecb = sm.tile([128, J], f32, tag="drecb")
                    nc.vector.tensor_copy(drecb[:], drec_ps[:])
                    # s = s_sum * (1/D[j]); drec carries a 64x factor from
                    # the fp16 D pre-scale, compensated by the 1/64 here
                    s_t = sm.tile([B, JS], f32, tag="st")
                    nc.vector.scalar_tensor_tensor(
                        out=s_t[:].rearrange("p (a b) -> p a b", b=S),
                        in0=s_sum[:].rearrange("p (a b) -> p a b", b=S),
                        scalar=1.0 / 64.0,
                        in1=drecb[:].unsqueeze(-1).broadcast_to([B, J, S]),
                        op0=AT.mult,
                        op1=AT.mult,
                    )

                # ---- squash (norm over J axis!) ----
                sq = sm.tile([B, JS], f32, tag="sq")
                nc.scalar.square(sq[:], s_t[:])
                msq = sm.tile([B, S], f32, tag="msq")
                nc.vector.tensor_reduce(
                    msq[:], sq[:].rearrange("p (a b) -> p b a", b=S),
                    axis=AX.X, op=AT.add)
                rsq = sm.tile([B, S], f32, tag="rsq")
                nc.scalar.sqrt(rsq[:], msq[:])
                # pre-warm the ACT exp table (real exp comes after the
                # b-update; ACT is idle during the A-matmuls)
                if not last:
                    nc.scalar.activation(dumo[:], dumi[:], ACT.Exp)
                den = sm.tile([B, S], f32, tag="den")
                nc.vector.tensor_scalar_add(den[:], msq[:], 1.0)
                rec = sm.tile([B, S], f32, tag="rec")
                nc.vector.reciprocal(rec[:], den[:])
                fmul = sm.tile([B, S], f32, tag="fmul")
                nc.vector.tensor_mul(fmul[:], rsq[:], rec[:])
                v_t = sm.tile([B, JS], f32 if last else mdt,
                              tag=f"vt{int(last)}")
                nc.vector.tensor_tensor(
                    out=v_t[:].rearrange("p (a b) -> p a b", b=S),
                    in0=s_t[:].rearrange("p (a b) -> p a b", b=S),
                    in1=fmul[:].unsqueeze(1).broadcast_to([B, J, S]),
                    op=AT.mult,
                )

                if last:
                    nc.sync.dma_start(v_d[:], v_t[:])
                    break

                # ---- b update: A = x^T v per (u,cr), drained by ACT ----
                for cr in range(NCR):
                    for u2 in range(U // 2):
                        a_ps = ps_a.tile([128, 2, JS], f32, tag="aps")
                        for h in range(2):
                            nc.tensor.matmul(a_ps[:, h],
                                             xa[:, cr, 2 * u2 + h],
                                             v_t[:], start=True, stop=True)
                        sl = slice(2 * u2, 2 * u2 + 2)
                        nc.scalar.copy(
                            a_sb[:, cr, sl].rearrange("p a b c -> p a (b c)"),
                            a_ps[:])
                # per-cr: one big mult, one fused (u,s) reduce, b += , then
                # e/dpart/ww for the next iteration, so cr0's scaled W (and
                # its s-matmuls) are ready while cr1's b-update still runs
                dpart_ps = ps_t.tile([1, NCR * J], f32, tag="tiny",
                                     name="dpart_ps")
                for cr in range(NCR):
                    nc.vector.tensor_tensor(
                        out=m_sb[:, cr], in0=wa[:, cr],
                        in1=a_sb[:, cr], op=AT.mult)
                    nc.vector.tensor_reduce(
                        binc_cr[cr][:],
                        m_sb[:, cr].rearrange("p u j s -> p j u s"),
                        axis=AX.XY, op=AT.add)
                    if first:
                        nc.vector.tensor_scalar_mul(
                            b_cr[cr][:], binc_cr[cr][:], 1.0 / B)
                    else:
                        nc.vector.scalar_tensor_tensor(
                            out=b_cr[cr][:], in0=binc_cr[cr][:],
                            scalar=1.0 / B, in1=b_cr[cr][:],
                            op0=AT.mult, op1=AT.add)
                    e_t = sm.tile([128, J], mdt, tag=f"e{cr}")
                    nc.scalar.activation(e_t[:], b_cr[cr][:], ACT.Exp)
                    nc.tensor.matmul(dpart_ps[:, cr * J:(cr + 1) * J],
                                     ones[:], e_t[:],
                                     start=True, stop=True)
                    if cr == NCR - 1:
                        nc.scalar.mul(dpart[:], dpart_ps[:], 1.0 / 64.0)
                    e_bch = (e_t[:].unsqueeze(1).unsqueeze(-1)
                             .broadcast_to([128, U // 2, J, S]))
                    for hh in range(2):
                        sl2 = slice(4 * hh, 4 * hh + 4)
                        nc.vector.tensor_tensor(
                            out=ww[:, cr, sl2], in0=wa[:, cr, sl2],
                            in1=e_bch, op=AT.mult)

    nc.compile()
    return nc


def _shard_inputs(x, W, mmdt="bf16"):
    if mmdt == "bf16":
        import ml_dtypes
        cast = lambda a: np.ascontiguousarray(a, dtype=ml_dtypes.bfloat16)
    else:
        cast = lambda a: np.ascontiguousarray(a, dtype=np.float32)
    x = np.ascontiguousarray(x, dtype=np.float32)
    W = np.ascontiguousarray(W, dtype=np.float32)
    in_maps = []
    for m in range(N_CORES):
        xc = x[:, :, m * C_LOC:(m + 1) * C_LOC]          # [B, U, 256]
        xr = xc.reshape(B, U, NCR, 128)                  # c_loc -> (cr, p)
        xs = cast(xr.transpose(3, 2, 1, 0))              # [128,NCR,U,B]
        xa = cast(xr.transpose(0, 2, 1, 3))              # [B,NCR,U,128]
        Wc = W[0, m * C_LOC:(m + 1) * C_LOC]             # [256, J, S, U]
        wr = Wc.reshape(NCR, 128, J, S, U)
        wa = cast(wr.transpose(1, 0, 4, 2, 3))           # [128,NCR,U,J,S]
        in_maps.append({"xs": xs, "xa": xa, "wa": wa})
    return in_maps


MMDT = "bf16"


def run(x, W, trace=False):
    from concourse import bass_utils

    if "nc" not in _cache:
        _cache["nc"] = _build(mmdt=MMDT)
    nc = _cache["nc"]
    in_maps = _shard_inputs(x, W, mmdt=MMDT)
    res = bass_utils.run_bass_kernel_spmd(
        nc, in_maps, core_ids=list(range(N_CORES)), trace=trace)
    v = res.results[0]["v"].reshape(B, J, S, 1).astype(np.float32)
    return v, res


def kernel(x, W):
    v, _ = run(x, W)
    return v

